# Initial kernel scaffold
#
"""Trainium2 Bass kernel for blocked-modality BertSelfAttention.

Problem: inputs [4, 2048, 768]; per-head QKV weights [12, 256, 64] where head h
uses modality block m = h // 4 of the hidden dim (768 = 3 x 256); standard
softmax attention per head; output [4, 2048, 768] (heads concatenated).

Sharding: 8 cores = 4 batches x 2 head-groups (heads 0-5 / 6-11). Each core
computes 6 heads = 3 head pairs, where both heads of a pair share a modality
slice of x. All data-layout prep (transpose of x, per-pair weight concat) is
done host-side; all FLOPs (projections, scores, softmax, context, bias adds,
normalization) run on-device.

Per-core device pipeline (all matmuls fp32r = full PE rate at free-dim >= 256):
  - Q^T/K^T proj: lhsT = [Wq_A | Wq_B] (128x128), rhs = x^T -> psum [128, S]
    rows 0-63 = head A, 64-127 = head B; bias added during PSUM->SBUF evac.
  - V proj: lhsT = x^T chunk, rhs = [Wv_A|Wv_B|dup] (N=256 to stay at full
    rate); stored as [t, 65] blocks [V_h | ones] for the fused context matmul.
  - scores^T [t, s] per t-tile: two concurrent matmuls (row groups 0-63 /
    64-127, auto tile_position) -> one PSUM tile; exp(0.125 * scores) on ACT
    in large ops (no max subtraction: |scores| < ~1 by construction).
  - context: ctx^T/sums fused: lhsT = [V_h | ones] [128, 65], rhs = P^T
    accumulating over the 16 t-tiles in PSUM [65, S-chunk].
  - epilogue per 128-row s-block: PE transpose of the [66, 128] ctx block
    (rows 0-63 ctx, row 64 sums), then one fused DVE op computes
    out = ctx_T * recip(sums) + bv.
"""

import sys

for _p in ("/opt/trn_rl_repo",):
    if _p not in sys.path:
        sys.path.insert(0, _p)

import numpy as np

import concourse.bass as bass
import concourse.mybir as mybir
from concourse import bacc, masks
from concourse.bass_utils import run_bass_kernel_spmd
from concourse.tile import TileContext

F32 = mybir.dt.float32
F32R = mybir.dt.float32r

# ---- custom DVE op: exp(x * 0.125) via (1 + x(c1 + x(c2 + x c3)))^4 ----
# Valid for |x| <= 8 (raw scores are well within), rel err <= 2e-4 -- on par
# with the fp32r (FP22) matmul rounding. Lets the softmax exp run on the
# Vector engine in parallel with the Scalar engine's table-based exp.
import re as _re

from concourse.dve_ops import OPS as _DVE_OPS
from concourse.dve_ops import (
    CUSTOM_DVE_SPECS as _DVE_SPECS,
    _CUSTOM_DVE_ROW_BASE,
    _SUB_OPCODE_FOR_NAME,
    DveOp as _DveOp,
)
from concourse.dve_spec import C0, C1, C2, One, Spec, Src0, sq

EXP4_C = (0.031252119457521335, 0.0004900867090307086, 5.042148435554105e-06)


def _exp4_ref(in0, in1, s0, s1, imm2):
    q = 1.0 + in0 * (s0 + in0 * (s1 + in0 * imm2))
    return ((q * q) ** 2).astype(np.float32)


def _register_exp4():
    name = "EXP4_POLY_ANT"
    t = (Src0 * C2 + C1) * Src0 + C0
    spec = Spec(body=sq(sq(t * Src0 + One)), reference=_exp4_ref)
    if name in _SUB_OPCODE_FOR_NAME:
        for op in _DVE_OPS:
            if op.name == name:
                return op
    probe = _DveOp(name, spec, subdim=False, uops_sha={})
    _DVE_OPS.append(probe)
    _SUB_OPCODE_FOR_NAME[name] = _CUSTOM_DVE_ROW_BASE + len(_DVE_OPS) - 1
    _DVE_SPECS[name] = spec
    from concourse.dve_table_gen import dve_ver_for
    ver = dve_ver_for("TRN2")
    try:
        probe.compile(ver)
        op = probe
    except ValueError as e:
        m = _re.search(r"v\d: ([0-9a-f]+)", str(e)) or _re.search(
            r"\(([0-9a-f]{8,})", str(e))
        if m is None:
            raise
        op = _DveOp(name, spec, subdim=False, uops_sha={ver: m.group(1)})
        _DVE_OPS[-1] = op
    return op


EXP4_OP = _register_exp4()

B, S, HID = 4, 2048, 768
H, DBLK, HD = 12, 256, 64
NCORES = 8
HPC = 6           # heads per core
NPAIR = 3         # head pairs per core
SCH = 512         # s-chunk width for the attention inner loop
NSCH = S // SCH   # 4
NT = S // 128     # 16 t-tiles (key blocks)
NSB = S // 128    # 16 output s-blocks

# t-tile groups for the scores/exp pipeline: (count, which psum pool)
T_GROUPS = [(2, "big"), (1, "sml")] * 5 + [(1, "big")]
assert sum(g[0] for g in T_GROUPS) == NT


def build_bass():
    nc = bacc.Bacc(None, target_bir_lowering=False)

    xt = nc.dram_tensor("xt", [NPAIR, 2, 128, S], F32R, kind="ExternalInput")
    wq = nc.dram_tensor("wq", [NPAIR, 2, 128, 128], F32R, kind="ExternalInput")
    wk = nc.dram_tensor("wk", [NPAIR, 2, 128, 128], F32R, kind="ExternalInput")
    wv = nc.dram_tensor("wv", [NPAIR, 2, 128, 256], F32R, kind="ExternalInput")
    bqk = nc.dram_tensor("bqk", [128, 2, NPAIR], F32, kind="ExternalInput")
    bv = nc.dram_tensor("bv", [NPAIR, 2, 64], F32, kind="ExternalInput")
    out = nc.dram_tensor("out", [S, HPC * HD], F32, kind="ExternalOutput")

    with TileContext(nc) as tc:
        with (
            tc.tile_pool(name="const", bufs=1) as cpool,
            tc.tile_pool(name="xts", bufs=2) as xtpool,
            tc.tile_pool(name="pt", bufs=10) as ppool,
            tc.tile_pool(name="cs", bufs=3) as cspool,
            tc.tile_pool(name="rcp", bufs=8) as rcpool,
            tc.tile_pool(name="ps_sc", bufs=4, space="PSUM") as pssc,
            tc.tile_pool(name="ps_ctx", bufs=1, space="PSUM") as psc,
            tc.tile_pool(name="ps_x", bufs=2, space="PSUM") as psx,
        ):
            # ---- constant/staging tiles (whole-kernel lifetime) ----
            wq_sb = cpool.tile([128, NPAIR, 2, 128], F32R)
            wk_sb = cpool.tile([128, NPAIR, 2, 128], F32R)
            wv_sb = cpool.tile([128, NPAIR, 2, 256], F32R)
            bqk_sb = cpool.tile([128, 2, NPAIR], F32)
            id66 = cpool.tile([66, 66], F32R)
            id66_32 = cpool.tile([66, 66], F32)
            bvb = cpool.tile([128, HPC, 64], F32)
            oz32 = cpool.tile([128, NPAIR, NT, 2, 2], F32)
            out_stage = cpool.tile([128, NSB, HPC * HD], F32)
            q_all = cpool.tile([128, NPAIR, S], F32R)
            k_all = cpool.tile([128, NPAIR, S], F32R)
            v_all = cpool.tile([128, NPAIR, NT, 132], F32R)

            # ---- projection emitter (interleaved with attention below) ----
            def proj_slice(p, xt_sb, sc):
                """Emit the proj work for pair p, slice sc: q/k chunk sc plus
                v t-tiles [4*sc, 4*sc+4)."""
                ps_q = psx.tile([128, SCH], F32, tag="x", name="ps_q")
                for dc in range(2):
                    nc.tensor.matmul(
                        ps_q[:, :],
                        wq_sb[:, p, dc, :],
                        xt_sb[:, dc, sc * SCH:(sc + 1) * SCH],
                        start=(dc == 0), stop=(dc == 1),
                    )
                nc.vector.tensor_scalar_add(
                    q_all[:, p, sc * SCH:(sc + 1) * SCH], ps_q[:, :],
                    bqk_sb[:, 0, p:p + 1])
                ps_k = psx.tile([128, SCH], F32, tag="x", name="ps_k")
                for dc in range(2):
                    nc.tensor.matmul(
                        ps_k[:, :],
                        wk_sb[:, p, dc, :],
                        xt_sb[:, dc, sc * SCH:(sc + 1) * SCH],
                        start=(dc == 0), stop=(dc == 1),
                    )
                nc.vector.tensor_scalar_add(
                    k_all[:, p, sc * SCH:(sc + 1) * SCH], ps_k[:, :],
                    bqk_sb[:, 1, p:p + 1])
                # pair 0's V matmul carries pairs 0 AND 1 (same modality,
                # host packs [Vp0A|Vp0B|Vp1A|Vp1B]); pair 1 emits no V work.
                if p == 1:
                    return
                for t in range(4 * sc, 4 * sc + 4):
                    ps_v = psx.tile([128, 256], F32, tag="x", name="ps_v")
                    for dc in range(2):
                        nc.tensor.matmul(
                            ps_v[:, :],
                            xt_sb[:, dc, t * 128:(t + 1) * 128],
                            wv_sb[:, p, dc, :],
                            start=(dc == 0), stop=(dc == 1),
                        )
                    if p == 0:
                        nc.vector.tensor_copy(
                            v_all[:, 0:2, t, :].rearrange(
                                "q r (h e) -> q r h e", h=2)[:, :, :, 0:64],
                            ps_v[:, 0:256].rearrange(
                                "q (r h e) -> q r h e", r=2, h=2),
                        )
                    else:
                        nc.vector.tensor_copy(
                            v_all[:, p, t, 0:132].rearrange(
                                "q (h e) -> q h e", h=2)[:, :, 0:64],
                            ps_v[:, 0:128].rearrange("q (h e) -> q h e", h=2),
                        )

            def load_xt(p, s_lo=0, s_hi=S):
                # pair 0 is loaded in two stages (first s-chunk before the
                # weight DMAs, the rest after) so the first projection starts
                # as early as possible; later pairs load in one DMA.
                xt_sb = xtpool.tile([128, 2, S], F32R, tag="xt", name="xt_sb")
                nc.sync.dma_start(
                    out=xt_sb[:, :, s_lo:s_hi],
                    in_=xt[p, :, :, s_lo:s_hi].rearrange("c q s -> q c s"))
                return xt_sb

            def load_xt_rest(p, xt_sb, s_lo):
                nc.sync.dma_start(
                    out=xt_sb[:, :, s_lo:],
                    in_=xt[p, :, :, s_lo:].rearrange("c q s -> q c s"))

            # pair 0: first s-chunk t-loop interleaved with its own proj
            xt_cur = load_xt(0, 0, SCH)

            nc.sync.dma_start(out=bqk_sb[:, :, :], in_=bqk[:, :, :])
            nc.sync.dma_start(out=wq_sb[:, :, :, :],
                              in_=wq.rearrange("p c q m -> q p c m"))
            nc.sync.dma_start(out=wk_sb[:, :, :, :],
                              in_=wk.rearrange("p c q m -> q p c m"))
            load_xt_rest(0, xt_cur, SCH)
            nc.sync.dma_start(out=wv_sb[:, :, :, :],
                              in_=wv.rearrange("p c q m -> q p c m"))

            # transpose identity (f32 staging -> f32r) + per-head bv broadcast
            masks.make_identity(nc, id66_32[:, :])
            nc.vector.tensor_copy(id66[:, :], id66_32[:, :])
            for hl in range(HPC):
                nc.sync.dma_start(
                    out=bvb[:, hl, :],
                    in_=bass.AP(bv, hl * 64, [[0, 128], [1, 64]]))
            nc.vector.memset(oz32[:, :, :, :, 0:1], 1.0)
            nc.vector.memset(oz32[:, :, :, :, 1:2], 0.0)
            nc.vector.tensor_copy(
                v_all[:, :, :, :].rearrange(
                    "q p t (h e) -> q p t h e", h=2)[:, :, :, :, 64:66],
                oz32[:, :, :, :, :],
            )


            def emit_tloop(p, sc, interleave_proj0=False):
                ctx_ps = psc.tile([66, 2 * SCH], F32, tag="c", name="ctx_ps")
                pend_ctx = []

                def emit_ctx(t, ph):
                    nc.tensor.matmul(
                        ctx_ps[:, 0:SCH],
                        v_all[:, p, t, 0:66],
                        ph[0][:, :],
                        start=(t == 0), stop=(t == NT - 1),
                    )
                    nc.tensor.matmul(
                        ctx_ps[:, SCH:2 * SCH],
                        v_all[:, p, t, 66:132],
                        ph[1][:, :],
                        start=(t == 0), stop=(t == NT - 1),
                    )

                for t in range(NT):
                    if interleave_proj0 and t % 4 == 0:
                        proj_slice(0, xt_cur, t // 4)
                    # per-head score tiles: 4 independent 1-bank slots halve
                    # the scores->exp->release chain granularity
                    halves = []
                    for hh in range(2):
                        sc_ps = pssc.tile([128, 512], F32, tag="sc", name="sc_ps")
                        nc.tensor.matmul(
                            sc_ps[:, :],
                            k_all[hh * 64:hh * 64 + 64, p, t * 128:(t + 1) * 128],
                            q_all[hh * 64:hh * 64 + 64, p, sc * SCH:(sc + 1) * SCH],
                            start=True, stop=True,
                        )
                        halves.append(sc_ps)
                    ph = []
                    for hh in range(2):
                        p_sb = ppool.tile([128, 512], F32R, tag="pt", name="p_sb")
                        if (2 * t + hh) % 16 in (1, 3, 5, 7, 10, 12, 14):
                            nc.vector._custom_dve(
                                EXP4_OP, out=p_sb[:, :], in0=halves[hh][:, :],
                                s0=EXP4_C[0], s1=EXP4_C[1], imm2=EXP4_C[2])
                        else:
                            nc.scalar.activation(
                                p_sb[:, :], halves[hh][:, :],
                                mybir.ActivationFunctionType.Exp, scale=0.125)
                        ph.append(p_sb)
                    # defer the ctx matmuls: the next scores pair refills the
                    # just-released psum slots before PE turns to ctx
                    pend_ctx.append((t, ph))
                    if len(pend_ctx) > 2:
                        emit_ctx(*pend_ctx.pop(0))
                for args in pend_ctx:
                    emit_ctx(*args)
                ctx_sb = cspool.tile([66, 2 * SCH], F32R, tag="cs", name="ctx_sb")
                nc.scalar.copy(ctx_sb[:, 0:SCH], ctx_ps[:, 0:SCH])
                nc.scalar.copy(ctx_sb[:, SCH:2 * SCH], ctx_ps[:, SCH:2 * SCH])
                return ctx_sb

            def emit_epilogue(p, sc, ctx_sb):
                for hh in range(2):
                    hl = 2 * p + hh
                    for blk in range(SCH // 128):
                        sb = sc * (SCH // 128) + blk
                        fin = psx.tile([128, 66], F32R, tag="x", name="fin")
                        nc.tensor.transpose(
                            fin[:, :],
                            ctx_sb[:, hh * SCH + blk * 128:
                                   hh * SCH + (blk + 1) * 128],
                            id66[:, :],
                        )
                        rc = rcpool.tile([128, 1], F32, tag="rc")
                        nc.vector.reciprocal(rc[:, :], fin[:, 64:65])
                        nc.vector.scalar_tensor_tensor(
                            out_stage[:, sb, hl * 64:(hl + 1) * 64],
                            fin[:, 0:64], rc[:, :], bvb[:, hl, :],
                            mybir.AluOpType.mult, mybir.AluOpType.add)
                nc.sync.dma_start(
                    out=out.rearrange("(t q) c -> q t c", q=128)[
                        :, sc * 4:(sc + 1) * 4, p * 2 * HD:(p + 1) * 2 * HD],
                    in_=out_stage[:, sc * 4:(sc + 1) * 4,
                                  p * 2 * HD:(p + 1) * 2 * HD],
                )

            # attention for pair p, interleaved with proj for pair p+1;
            # the (fins + out-DMA) epilogue trails one s-chunk behind so the
            # next s-chunk's scores keep the PE/ACT pipeline fed first.
            pending = None
            xt_nxt = None
            for p in range(NPAIR):
                if p + 1 < NPAIR:
                    xt_nxt = load_xt(p + 1)
                for sc in range(NSCH):
                    ctx_sb = emit_tloop(p, sc, interleave_proj0=(p == 0 and sc == 0))
                    if pending is not None:
                        emit_epilogue(*pending)
                    if p + 1 < NPAIR:
                        proj_slice(p + 1, xt_nxt, sc)
                    pending = (p, sc, ctx_sb)
            emit_epilogue(*pending)

    nc.finalize()
    return nc


_NC = None


def _get_nc():
    global _NC
    if _NC is None:
        _NC = build_bass()
    return _NC


def _prep_core_inputs(inputs, Wq, bq, Wk, bk, Wv, bv, b, g):
    heads = list(range(g * HPC, (g + 1) * HPC))
    # order pairs so pairs 0 and 1 share a modality (device packs their V
    # projection into one matmul pass)
    pairs = [(heads[0], heads[1]), (heads[2], heads[3]), (heads[4], heads[5])]
    if pairs[0][0] // 4 != pairs[1][0] // 4:
        pairs = [pairs[1], pairs[2], pairs[0]]
    assert pairs[0][0] // 4 == pairs[1][0] // 4
    head_order = [h for pr in pairs for h in pr]
    xT = np.ascontiguousarray(inputs[b].T)  # [HID, S]

    xt = np.empty((NPAIR, 2, 128, S), np.float32)
    wq_h = np.empty((NPAIR, 2, 128, 128), np.float32)
    wk_h = np.empty((NPAIR, 2, 128, 128), np.float32)
    wv_h = np.zeros((NPAIR, 2, 128, 256), np.float32)
    bqk_h = np.empty((2, NPAIR, 128), np.float32)
    bv_h = np.empty((NPAIR, 2, 64), np.float32)

    for p in range(NPAIR):
        hA, hB = pairs[p]
        mod = hA // 4
        assert hB // 4 == mod
        for dc in range(2):
            d0 = mod * DBLK + dc * 128
            xt[p, dc] = xT[d0:d0 + 128]
            wq_h[p, dc] = np.concatenate(
                [Wq[hA][dc * 128:(dc + 1) * 128], Wq[hB][dc * 128:(dc + 1) * 128]], axis=1)
            wk_h[p, dc] = np.concatenate(
                [Wk[hA][dc * 128:(dc + 1) * 128], Wk[hB][dc * 128:(dc + 1) * 128]], axis=1)
        bqk_h[0, p] = np.concatenate([bq[hA], bq[hB]])
        bqk_h[1, p] = np.concatenate([bk[hA], bk[hB]])
        bv_h[p, 0] = bv[hA]
        bv_h[p, 1] = bv[hB]

    # V weights: pair-0 slot carries pairs 0 and 1 (same modality);
    # pair-1 slot is unused; pair 2 duplicated to keep N=256.
    for dc in range(2):
        wv_h[0, dc] = np.concatenate(
            [Wv[pairs[0][0]][dc * 128:(dc + 1) * 128],
             Wv[pairs[0][1]][dc * 128:(dc + 1) * 128],
             Wv[pairs[1][0]][dc * 128:(dc + 1) * 128],
             Wv[pairs[1][1]][dc * 128:(dc + 1) * 128]], axis=1)
        w2 = np.concatenate(
            [Wv[pairs[2][0]][dc * 128:(dc + 1) * 128],
             Wv[pairs[2][1]][dc * 128:(dc + 1) * 128]], axis=1)
        wv_h[2, dc] = np.concatenate([w2, w2], axis=1)

    bqk_dev = np.ascontiguousarray(bqk_h.transpose(2, 0, 1))
    return ({"xt": xt, "wq": wq_h, "wk": wk_h, "wv": wv_h,
             "bqk": bqk_dev, "bv": bv_h}, head_order)


def run_cores(inputs, Wq, bq, Wk, bk, Wv, bv, **kwargs):
    """Build per-core shards, run on 8 NeuronCores, return (full_out, results)."""
    args = [np.asarray(a, np.float32) for a in (inputs, Wq, bq, Wk, bk, Wv, bv)]
    inputs, Wq, bq, Wk, bk, Wv, bv = args
    in_maps = []
    orders = []
    for core in range(NCORES):
        b, g = core // 2, core % 2
        m, order = _prep_core_inputs(inputs, Wq, bq, Wk, bk, Wv, bv, b, g)
        in_maps.append(m)
        orders.append(order)
    nc = _get_nc()
    res = run_bass_kernel_spmd(nc, in_maps, core_ids=list(range(NCORES)), **kwargs)
    full = np.empty((B, S, H * HD), np.float32)
    for core in range(NCORES):
        b = core // 2
        o = res.results[core]["out"]
        for hl, h in enumerate(orders[core]):
            full[b, :, h * HD:(h + 1) * HD] = o[:, hl * HD:(hl + 1) * HD]
    return full, res


def kernel(inputs, Wq, bq, Wk, bk, Wv, bv):
    full, _ = run_cores(inputs, Wq, bq, Wk, bk, Wv, bv)
    return full



# revision 93
# speedup vs baseline: 1.2612x; 1.2612x over previous
"""Trainium2 Bass kernel for blocked-modality BertSelfAttention.

Problem: inputs [4, 2048, 768]; per-head QKV weights [12, 256, 64] where head h
uses modality block m = h // 4 of the hidden dim (768 = 3 x 256); standard
softmax attention per head; output [4, 2048, 768] (heads concatenated).

Sharding: 8 cores = 4 batches x 2 head-groups (heads 0-5 / 6-11). Each core
computes 6 heads = 3 head pairs, where both heads of a pair share a modality
slice of x.

Cost-model-driven design (PE cost = out_free_size x cycles_per_row, serialized
per engine; fp8 DoubleRow = 0.5 cy/row; GPSIMD cannot touch PSUM, so all psum
evacuations live on ACT/DVE alongside the exp ops):
  - ONE flat software-pipelined stream over all 12 (pair, s-chunk) x 16 t
    steps: scores -> exp -> deferred ctx, with projections and epilogues
    interleaved as scheduled "pieces" so no engine ever drains at a boundary.
  - Q/K projections: fp8e4 DoubleRow (contraction 256 = 2x128 subtiles in one
    instruction), weights/x pre-quantized host-side with a x32 scale folded
    into Wq/Wk (1/1024 compensated in the exp scale); bq is added via a
    rank-1 ones (x) bq matmul; bk is dropped entirely (softmax is invariant
    to per-query constants).
  - scores^T per (head, t): ONE fp8 DoubleRow matmul [32part x 2ksub
    contraction] -> psum [128 keys, 512 q] at 256 cycles; q/k stored packed
    [64, pair, ksub, q/k, S] fp8, head A at partitions 0-31, B at 32-63.
  - exp: [128, 1024] ops (both heads of a t-tile in one 2-bank psum tile),
    strictly alternating ACT (table exp) / DVE (custom poly EXP4), writing
    fp8 probs into [128, 2tsub, 1024] t-pair tiles.
  - ctx^T: ONE fp8 DoubleRow matmul per (head, t-PAIR): lhsT = [V+bv | 1 | 0]
    [128, 2tsub, 66] fp8, rhs = P [128, 2tsub, 512] fp8 -> [66, 512] psum at
    256 cycles; row 64 accumulates the softmax denominators, bv is folded
    into V by a rank-1 matmul at projection time so the epilogue is a pure
    divide.
  - epilogue: ACT+DVE drain [ctx^T; sums] to bf16 SBUF, ONE xbar DMA
    transpose [80, 1024] -> [128, 8, 80], ONE Pool broadcast-divide into the
    staging buffer, out-DMA from SP.
  - PSUM budget exactly 8 banks: 3 x [128,1024] scores slots (projection
    psum rides the same rotation) + 1 x [66, 1024] ctx slot.
"""

import sys

for _p in ("/opt/trn_rl_repo",):
    if _p not in sys.path:
        sys.path.insert(0, _p)

import ml_dtypes
import numpy as np

import concourse.bass as bass
import concourse.mybir as mybir
from concourse import bacc, masks
from concourse.bass_utils import run_bass_kernel_spmd
from concourse.tile import TileContext

F32 = mybir.dt.float32
F32R = mybir.dt.float32r
BF16 = mybir.dt.bfloat16
FP8 = mybir.dt.float8e4
DR = mybir.MatmulPerfMode.DoubleRow

NP_FP8 = mybir.dt.np(FP8)
NP_BF16 = mybir.dt.np(BF16)

# ---- custom DVE op: exp(x * s) via (1 + x(c1 + x(c2 + x c3)))^4 ----
# Valid for |s*x| <= 1 (raw scores are well within), rel err <= 2e-4. Lets
# the softmax exp run on the Vector engine in parallel with the Scalar
# engine's table-based exp.
import re as _re

from concourse.dve_ops import OPS as _DVE_OPS
from concourse.dve_ops import (
    CUSTOM_DVE_SPECS as _DVE_SPECS,
    _CUSTOM_DVE_ROW_BASE,
    _SUB_OPCODE_FOR_NAME,
    DveOp as _DveOp,
)
from concourse.dve_spec import C0, C1, C2, One, Spec, Src0, sq

EXP4_C = (0.031252119457521335, 0.0004900867090307086, 5.042148435554105e-06)


def _exp4_ref(in0, in1, s0, s1, imm2):
    q = 1.0 + in0 * (s0 + in0 * (s1 + in0 * imm2))
    return ((q * q) ** 2).astype(np.float32)


def _register_exp4():
    name = "EXP4_POLY_ANT"
    t = (Src0 * C2 + C1) * Src0 + C0
    spec = Spec(body=sq(sq(t * Src0 + One)), reference=_exp4_ref)
    if name in _SUB_OPCODE_FOR_NAME:
        for op in _DVE_OPS:
            if op.name == name:
                return op
    probe = _DveOp(name, spec, subdim=False, uops_sha={})
    _DVE_OPS.append(probe)
    _SUB_OPCODE_FOR_NAME[name] = _CUSTOM_DVE_ROW_BASE + len(_DVE_OPS) - 1
    _DVE_SPECS[name] = spec
    from concourse.dve_table_gen import dve_ver_for
    ver = dve_ver_for("TRN2")
    try:
        probe.compile(ver)
        op = probe
    except ValueError as e:
        m = _re.search(r"v\d: ([0-9a-f]+)", str(e)) or _re.search(
            r"\(([0-9a-f]{8,})", str(e))
        if m is None:
            raise
        op = _DveOp(name, spec, subdim=False, uops_sha={ver: m.group(1)})
        _DVE_OPS[-1] = op
    return op


EXP4_OP = _register_exp4()

B, S, HID = 4, 2048, 768
H, DBLK, HD = 12, 256, 64
NCORES = 8
HPC = 6           # heads per core
NPAIR = 3         # head pairs per core
SCH = 512         # s-chunk width for the attention inner loop
NSCH = S // SCH   # 4
NT = S // 128     # 16 t-tiles (key blocks)
NSB = S // 128    # 16 output s-blocks

WS = 32.0                       # scale folded into Wq/Wk/bq host-side
EXP_SCALE = 0.125 / (WS * WS)   # exp scale applied to raw fp8 scores
EXP4_CS = tuple(c / (WS * WS) ** (i + 1) for i, c in enumerate(EXP4_C))

# ACT/DVE split for the 192 exp ops: strict alternation. Any biased
# pattern needs same-engine runs, and each run serializes two ~1us ops
# back-to-back, which costs more than the 1038 vs 1192ns per-op imbalance.
def exp_on_act(g):
    return (g * 7) % 13 < 7


import os
_SKIP_EPI = bool(int(os.environ.get('SKIP_EPI', '0')))
_SKIP_PIECES = bool(int(os.environ.get('SKIP_PIECES', '0')))


def build_bass():
    nc = bacc.Bacc(None, target_bir_lowering=False)

    xt8 = nc.dram_tensor("xt8", [NPAIR, 128, 2, S], FP8, kind="ExternalInput")
    xtv = nc.dram_tensor("xtv", [NPAIR, 2, 128, S], BF16, kind="ExternalInput")
    wq8 = nc.dram_tensor("wq8", [NPAIR, 128, 2, 128], FP8, kind="ExternalInput")
    wk8 = nc.dram_tensor("wk8", [NPAIR, 128, 2, 128], FP8, kind="ExternalInput")
    wv = nc.dram_tensor("wv", [NPAIR, 2, 128, 256], BF16, kind="ExternalInput")
    bqp = nc.dram_tensor("bqp", [NPAIR, 128], BF16, kind="ExternalInput")
    bvp = nc.dram_tensor("bvp", [NPAIR, 256], BF16, kind="ExternalInput")
    out = nc.dram_tensor("out", [S, HPC * HD], F32, kind="ExternalOutput")

    with TileContext(nc) as tc:
        with (
            tc.tile_pool(name="const", bufs=1) as cpool,
            tc.tile_pool(name="xt8s", bufs=2) as xt8pool,
            tc.tile_pool(name="xtvs", bufs=2) as xtvpool,
            tc.tile_pool(name="pt", bufs=6) as ppool,
            tc.tile_pool(name="cs", bufs=3) as cspool,
            tc.tile_pool(name="tr", bufs=3) as trpool,
            # 6 banks: depth-3 scores rotation; short-lived projection psum
            # tiles ride the same rotation (released by their evac long
            # before the slot comes around again)
            tc.tile_pool(name="ps_sc", bufs=3, space="PSUM") as pssc,
            # 2 banks: per-chunk ctx^T accumulators (one per head), then the
            # transpose outputs rotate through the same slots
            tc.tile_pool(name="ps_ctx", bufs=1, space="PSUM") as psc,
        ):
            # ---- constant/staging tiles (whole-kernel lifetime) ----
            wq_sb = cpool.tile([128, NPAIR, 2, 128], FP8)
            wk_sb = cpool.tile([128, NPAIR, 2, 128], FP8)
            wv_sb = cpool.tile([128, NPAIR, 2, 256], BF16)
            bq_sb = cpool.tile([1, NPAIR, 128], BF16)
            bvr_sb = cpool.tile([1, NPAIR, 256], BF16)
            ones1 = cpool.tile([1, SCH], BF16)
            # V+bv in fp8, keys x [t-pair, t-sub] split for the DoubleRow
            # ctx; col 64 = ones (softmax denominator accumulator)
            # tsub ahead of tp so the DoubleRow weight AP's tsub stride
            # (8*2*66 = 2112B) meets the 16B-alignment ISA restriction
            v8 = cpool.tile([128, NPAIR, 2, NT // 2, 2, 66], FP8)
            # q and k packed: [64, pair, dsub j, q/k, S]
            qk8 = cpool.tile([64, NPAIR, 2, 2, S], FP8)
            out_stage = cpool.tile([128, NSB, HPC * HD], F32)

            # ---- input DMAs for pair 0 + weights ----
            # pair-0 x loads split: the first s-chunk (and first V-group key
            # block) lands before the bulk so the bootstrap projections and
            # first scores start ~2-3us earlier
            xt8_cur = xt8pool.tile([128, 2, S], FP8, tag="xt8", name="xt8_sb")
            nc.sync.dma_start(out=xt8_cur[:, :, 0:SCH],
                              in_=xt8[0, :, :, 0:SCH])
            xtv_cur = xtvpool.tile([128, 2, S], BF16, tag="xtv", name="xtv_sb")
            nc.sync.dma_start(out=xtv_cur[:, :, 0:SCH],
                              in_=xtv[0, :, :, 0:SCH].rearrange(
                                  "c q s -> q c s"))
            nc.sync.dma_start(out=bq_sb[:, :, :], in_=bqp[:, :])
            nc.sync.dma_start(out=wq_sb[:, :, :, :],
                              in_=wq8.rearrange("p q c m -> q p c m"))
            nc.sync.dma_start(out=wk_sb[:, :, :, :],
                              in_=wk8.rearrange("p q c m -> q p c m"))
            nc.sync.dma_start(out=wv_sb[:, :, :, :],
                              in_=wv.rearrange("p c q m -> q p c m"))
            nc.sync.dma_start(out=bvr_sb[:, :, :], in_=bvp[:, :])
            nc.sync.dma_start(out=xt8_cur[:, :, SCH:2 * SCH],
                              in_=xt8[0, :, :, SCH:2 * SCH])
            nc.sync.dma_start(out=xtv_cur[:, :, SCH:2 * SCH],
                              in_=xtv[0, :, :, SCH:2 * SCH].rearrange(
                                  "c q s -> q c s"))
            nc.sync.dma_start(out=xt8_cur[:, :, 2 * SCH:],
                              in_=xt8[0, :, :, 2 * SCH:])
            nc.sync.dma_start(out=xtv_cur[:, :, 2 * SCH:],
                              in_=xtv[0, :, :, 2 * SCH:].rearrange(
                                  "c q s -> q c s"))
            nc.vector.memset(ones1[:, :], 1.0)
            nc.vector.memset(v8[:, :, :, :, :, 64:66], 0.0)
            nc.vector.memset(v8[:, :, :, :, :, 64:65], 1.0)

            # ---- projection emitters (Pool does all psum evacs); psum
            # tiles come from the shared pssc rotation, one full [128, 1024]
            # slot per emitter call to minimize rotation churn ----
            def emit_qk(p, sc, xt8_sb):
                # q chunk in [0:512] (with rank-1 +bq), k chunk in [512:1024]
                ps_qk = pssc.tile([128, 1024], F32, tag="sc", name="ps_qk")
                nc.tensor.matmul(
                    ps_qk[:, 0:SCH], wq_sb[:, p, :, :],
                    xt8_sb[:, :, sc * SCH:(sc + 1) * SCH],
                    start=True, stop=False, perf_mode=DR)
                nc.tensor.matmul(
                    ps_qk[:, 0:SCH], bq_sb[0:1, p, :], ones1[0:1, :],
                    start=False, stop=True)
                nc.tensor.matmul(
                    ps_qk[:, SCH:2 * SCH], wk_sb[:, p, :, :],
                    xt8_sb[:, :, sc * SCH:(sc + 1) * SCH],
                    start=True, stop=True, perf_mode=DR)
                # evac halves run CONCURRENTLY on ACT + Pool so the borrowed
                # scores-psum slot releases in half the time
                nc.scalar.copy(
                    qk8[0:64, p, 0, :, sc * SCH:(sc + 1) * SCH],
                    ps_qk[0:64, :].rearrange("q (w s) -> q w s", w=2))
                nc.scalar.copy(
                    qk8[0:64, p, 1, :, sc * SCH:(sc + 1) * SCH],
                    ps_qk[64:128, :].rearrange("q (w s) -> q w s", w=2))

            def emit_v4(p, g, xtv_sb):
                # V+bv for t-tiles 4g..4g+3 in one psum slot. p == 0 packs
                # pairs 0 AND 1 (same modality, 4 heads, N=256); p == 2 emits
                # its own 2 heads at N=128.
                n = 256 if p == 0 else 128
                ps_v = pssc.tile([128, 1024], F32, tag="sc", name="ps_v")
                for tt in range(4):
                    t = 4 * g + tt
                    for dc in range(2):
                        nc.tensor.matmul(
                            ps_v[:, tt * n:tt * n + n],
                            xtv_sb[:, dc, t * 128:(t + 1) * 128],
                            wv_sb[:, p, dc, 0:n],
                            start=(dc == 0), stop=False)
                    # rank-1 bias: V += 1 (x) bv, so the epilogue is a pure
                    # divide (out = sum(P(V+bv)) / sum(P))
                    nc.tensor.matmul(
                        ps_v[:, tt * n:tt * n + n],
                        ones1[0:1, 0:128], bvr_sb[0:1, p, 0:n],
                        start=False, stop=True)
                # evac halves run CONCURRENTLY on ACT + Pool so the borrowed
                # scores-psum slot releases in half the time
                if p == 0:
                    view = ps_v[:, :].rearrange(
                        "q (a b r h e) -> q r b a h e", a=2, b=2, r=2, h=2)
                    nc.scalar.copy(v8[:, 0, :, 2 * g:2 * g + 2, :, 0:64],
                                   view[:, 0])
                    nc.vector.tensor_copy(
                        v8[:, 1, :, 2 * g:2 * g + 2, :, 0:64], view[:, 1])
                else:
                    nc.vector.tensor_copy(
                        v8[:, 2, :, 2 * g:2 * g + 2, :, 0:64],
                        ps_v[:, 0:512].rearrange(
                            "q (a b h e) -> q b a h e", a=2, b=2, h=2))

            def emit_transposes(ctx_sbT):
                # ONE DMA-transpose per chunk: [80, 1024] -> [128 q, 8, 80]
                # (blocks 0-3 head A qb 0-3, 4-7 head B; col 64 = sums)
                ctx_tr = trpool.tile([128, 8, 80], BF16, tag="tr",
                                     name="ctx_tr")
                nc.sync.dma_start_transpose(ctx_tr[:, :, :], ctx_sbT[:, :])
                return ctx_tr

            def emit_divides(p, sc, ctx_tr):
                # ONE Pool broadcast-divide per chunk (out = ctx / sum),
                # then the out-DMA from the lightly-loaded SP queue
                tr_v = ctx_tr[:, :, :].rearrange("q (h b) e -> q h b e", h=2)
                rc = trpool.tile([128, 2, 4, 1], F32, tag="rc", name="rc")
                nc.vector.reciprocal(rc[:, :, :, :], tr_v[:, :, :, 64:65])
                nc.vector.tensor_tensor(
                    out_stage[:, sc * 4:(sc + 1) * 4,
                              2 * p * HD:(2 * p + 2) * HD].rearrange(
                        "q b (h e) -> q h b e", h=2),
                    tr_v[:, :, :, 0:64],
                    rc[:, :, :, :].broadcast_to([128, 2, 4, 64]),
                    mybir.AluOpType.mult)
                nc.sync.dma_start(
                    out=out.rearrange("(t q) c -> q t c", q=128)[
                        :, sc * 4:(sc + 1) * 4, p * 2 * HD:(p + 1) * 2 * HD],
                    in_=out_stage[:, sc * 4:(sc + 1) * 4,
                                  p * 2 * HD:(p + 1) * 2 * HD])

            # ---- main loop ----
            # pair 0 bootstrap: qk chunk 0 + V group 0 before its own t-loop
            emit_qk(0, 0, xt8_cur)
            emit_v4(0, 0, xtv_cur)

            # ---- one flat pipelined stream over all (pair, chunk, t) ----
            # No barriers at chunk or pair boundaries: deferred ctx matmuls,
            # epilogues, projections, and the next chunk's scores interleave
            # freely so the exp engines never drain.
            chunks = [(p, sc) for p in range(NPAIR) for sc in range(NSCH)]
            pieces = {}  # global t index -> [thunks]

            def add(gt, thunk):
                pieces.setdefault(gt, []).append(thunk)

            # Projection schedule. Pair 0's bootstrap: qk chunk 0 + V group 0
            # before the stream; V groups 1-3 and qk chunks 1-3 inside chunk
            # (0,0), each ahead of first use. Pair pn's piece-groups G_i
            # (pair 1: just qk_i -- its V rides pair 0's packed pass; pair
            # 2: v4_i + qk_i) run at host chunk (pn-1, i+1) for i<3 and
            # (pn, 0) for i=3.
            xbufs = {0: (xt8_cur, xtv_cur)}
            for g in range(1, 4):
                add(4 * g - 2, lambda g=g: emit_v4(0, g, xbufs[0][1]))
            for sc2 in range(1, NSCH):
                add(4 * sc2 - 3, lambda s=sc2: emit_qk(0, s, xbufs[0][0]))
            for pn in range(1, NPAIR):
                for i in range(4):
                    ci = 4 * (pn - 1) + i + 1 if i < 3 else 4 * pn
                    if pn == 2:
                        add(ci * NT + 4,
                            lambda n=pn, g=i: emit_v4(2, g, xbufs[n][1]))
                    add(ci * NT + 8,
                        lambda n=pn, s=i: emit_qk(n, s, xbufs[n][0]))

            ctx_tiles = {}
            pend = []

            def emit_ctx(ci, tp, p_sb):
                # one DoubleRow matmul per head covers t-tiles 2tp, 2tp+1:
                # ctx^T[65, 512] += [V+bv|1]^T @ P^T
                pch, scch = chunks[ci]
                if ci not in ctx_tiles:
                    ctx_tiles[ci] = psc.tile([66, 2 * SCH], F32, tag="c",
                                             name="ctx_t")
                ct = ctx_tiles[ci]
                for hh in range(2):
                    nc.tensor.matmul(
                        ct[:, hh * SCH:(hh + 1) * SCH],
                        v8[:, pch, :, tp, hh, :],
                        p_sb[:, :, hh * 512:(hh + 1) * 512],
                        start=(tp == 0), stop=(tp == NT // 2 - 1),
                        perf_mode=DR)
                if tp == NT // 2 - 1:
                    # chunk complete: drain [ctx^T; sums] to bf16 SBUF (rows
                    # 65-79 pad to the 16-row xbar transpose granularity),
                    # kick the transposes now; defer the divide tail into the
                    # next chunk's piece schedule so its transpose-wait never
                    # head-of-line-blocks Pool's queue
                    ctx_sbT = cspool.tile([80, 2 * SCH], BF16, tag="cs",
                                          name="ctx_sbT")
                    nc.vector.tensor_copy(ctx_sbT[0:66, 0:SCH],
                                          ct[:, 0:SCH])
                    nc.vector.tensor_copy(ctx_sbT[0:66, SCH:2 * SCH],
                                          ct[:, SCH:2 * SCH])
                    del ctx_tiles[ci]
                    if not _SKIP_EPI:
                        ctx_tr = emit_transposes(ctx_sbT)
                        emit_divides(pch, scch, ctx_tr)

            p_sb = None
            for gi in range(len(chunks) * NT):
                ci, t = gi // NT, gi % NT
                p, sc = chunks[ci]
                if t == 0 and sc == 0 and p + 1 < NPAIR:
                    xt8_nxt = xt8pool.tile([128, 2, S], FP8, tag="xt8",
                                           name="xt8_sb")
                    nc.sync.dma_start(out=xt8_nxt[:, :, :],
                                      in_=xt8[p + 1, :, :, :])
                    xtv_nxt = xtvpool.tile([128, 2, S], BF16, tag="xtv",
                                           name="xtv_sb")
                    nc.sync.dma_start(
                        out=xtv_nxt[:, :, :],
                        in_=xtv[p + 1, :, :, :].rearrange("c q s -> q c s"))
                    xbufs[p + 1] = (xt8_nxt, xtv_nxt)
                sc_ps = pssc.tile([128, 1024], F32, tag="sc", name="sc_ps")
                for hh in range(2):
                    nc.tensor.matmul(
                        sc_ps[:, hh * 512:(hh + 1) * 512],
                        qk8[hh * 32:hh * 32 + 32, p, :, 1,
                            t * 128:(t + 1) * 128],
                        qk8[hh * 32:hh * 32 + 32, p, :, 0,
                            sc * SCH:(sc + 1) * SCH],
                        start=True, stop=True, perf_mode=DR)
                if t % 2 == 0:
                    p_sb = ppool.tile([128, 2, 1024], FP8, tag="pt",
                                      name="p_sb")
                if exp_on_act(gi):
                    nc.scalar.activation(
                        p_sb[:, t % 2, :], sc_ps[:, :],
                        mybir.ActivationFunctionType.Exp, scale=EXP_SCALE)
                else:
                    nc.vector._custom_dve(
                        EXP4_OP, out=p_sb[:, t % 2, :], in0=sc_ps[:, :],
                        s0=EXP4_CS[0], s1=EXP4_CS[1], imm2=EXP4_CS[2])
                if t % 2 == 1:
                    pend.append((ci, t // 2, p_sb))
                    if len(pend) > 2:
                        emit_ctx(*pend.pop(0))
                for thunk in (() if _SKIP_PIECES else pieces.pop(gi, ())):
                    thunk()
            for args in pend:
                emit_ctx(*args)
            # drain pieces scheduled past the end (last chunk's divide tail)
            if not _SKIP_PIECES:
                for gi in sorted(pieces):
                    for thunk in pieces[gi]:
                        thunk()

    nc.finalize()
    return nc


_NC = None


def _get_nc():
    global _NC
    if _NC is None:
        _NC = build_bass()
    return _NC


def _prep_core_inputs(inputs, Wq, bq, Wk, bk, Wv, bv, b, g):
    heads = list(range(g * HPC, (g + 1) * HPC))
    # order pairs so pairs 0 and 1 share a modality (device packs their V
    # projection into one matmul pass)
    pairs = [(heads[0], heads[1]), (heads[2], heads[3]), (heads[4], heads[5])]
    if pairs[0][0] // 4 != pairs[1][0] // 4:
        pairs = [pairs[1], pairs[2], pairs[0]]
    assert pairs[0][0] // 4 == pairs[1][0] // 4
    head_order = [h for pr in pairs for h in pr]
    xT = np.ascontiguousarray(inputs[b].T)  # [HID, S]

    xt8_h = np.empty((NPAIR, 128, 2, S), NP_FP8)
    xtv_h = np.empty((NPAIR, 2, 128, S), NP_BF16)
    wq8_h = np.empty((NPAIR, 128, 2, 128), NP_FP8)
    wk8_h = np.empty((NPAIR, 128, 2, 128), NP_FP8)
    wv_h = np.zeros((NPAIR, 2, 128, 256), NP_BF16)
    bq_h = np.empty((NPAIR, 128), np.float32)
    bv_h = np.zeros((NPAIR, 256), np.float32)

    # packed fp8 W columns: c -> (j = c//64, head = A if c%64<32 else B,
    # d = j*32 + c%32)
    def pack_w(Wh, hA, hB):
        wp = np.empty((128, 2, 128), np.float32)
        for j in range(2):
            for dc in range(2):
                blk = slice(dc * 128, (dc + 1) * 128)
                wp[:, dc, j * 64:j * 64 + 32] = Wh[hA][blk, j * 32:j * 32 + 32]
                wp[:, dc, j * 64 + 32:j * 64 + 64] = \
                    Wh[hB][blk, j * 32:j * 32 + 32]
        return (wp * WS).astype(NP_FP8)

    for p in range(NPAIR):
        hA, hB = pairs[p]
        mod = hA // 4
        assert hB // 4 == mod
        xs = xT[mod * DBLK:(mod + 1) * DBLK]  # [256, S]
        xt8_h[p] = xs.reshape(2, 128, S).transpose(1, 0, 2).astype(NP_FP8)
        xtv_h[p] = xs.reshape(2, 128, S).astype(NP_BF16)
        wq8_h[p] = pack_w(Wq, hA, hB)
        wk8_h[p] = pack_w(Wk, hA, hB)
        # bq packed to match the wq column order (added via rank-1 matmul)
        for j in range(2):
            bq_h[p, j * 64:j * 64 + 32] = WS * bq[hA][j * 32:j * 32 + 32]
            bq_h[p, j * 64 + 32:j * 64 + 64] = WS * bq[hB][j * 32:j * 32 + 32]

    # bv rows match the wv column packing: row 0 covers pairs 0+1's packed
    # pass, row 2 covers pair 2's 128-wide pass
    bv_h[0] = np.concatenate([bv[pairs[0][0]], bv[pairs[0][1]],
                              bv[pairs[1][0]], bv[pairs[1][1]]])
    bv_h[2, 0:128] = np.concatenate([bv[pairs[2][0]], bv[pairs[2][1]]])

    # V weights: pair-0 slot carries pairs 0 and 1 (same modality);
    # pair-2 slot uses its first 128 columns only.
    for dc in range(2):
        wv_h[0, dc] = np.concatenate(
            [Wv[pairs[0][0]][dc * 128:(dc + 1) * 128],
             Wv[pairs[0][1]][dc * 128:(dc + 1) * 128],
             Wv[pairs[1][0]][dc * 128:(dc + 1) * 128],
             Wv[pairs[1][1]][dc * 128:(dc + 1) * 128]],
            axis=1).astype(NP_BF16)
        wv_h[2, dc, :, 0:128] = np.concatenate(
            [Wv[pairs[2][0]][dc * 128:(dc + 1) * 128],
             Wv[pairs[2][1]][dc * 128:(dc + 1) * 128]],
            axis=1).astype(NP_BF16)

    return ({"xt8": xt8_h, "xtv": xtv_h, "wq8": wq8_h, "wk8": wk8_h,
             "wv": wv_h, "bqp": bq_h.astype(NP_BF16),
             "bvp": bv_h.astype(NP_BF16)}, head_order)


def run_cores(inputs, Wq, bq, Wk, bk, Wv, bv, **kwargs):
    """Build per-core shards, run on 8 NeuronCores, return (full_out, results)."""
    args = [np.asarray(a, np.float32) for a in (inputs, Wq, bq, Wk, bk, Wv, bv)]
    inputs, Wq, bq, Wk, bk, Wv, bv = args
    in_maps = []
    orders = []
    for core in range(NCORES):
        b, g = core // 2, core % 2
        m, order = _prep_core_inputs(inputs, Wq, bq, Wk, bk, Wv, bv, b, g)
        in_maps.append(m)
        orders.append(order)
    nc = _get_nc()
    res = run_bass_kernel_spmd(nc, in_maps, core_ids=list(range(NCORES)), **kwargs)
    full = np.empty((B, S, H * HD), np.float32)
    for core in range(NCORES):
        b = core // 2
        o = res.results[core]["out"]
        for hl, h in enumerate(orders[core]):
            full[b, :, h * HD:(h + 1) * HD] = o[:, hl * HD:(hl + 1) * HD]
    return full, res


def kernel(inputs, Wq, bq, Wk, bk, Wv, bv):
    full, _ = run_cores(inputs, Wq, bq, Wk, bk, Wv, bv)
    return full
